# revision 1
# baseline (speedup 1.0000x reference)
"""Trainium2 Bass kernel for a 2-layer spiking (LIF) network.

Reference semantics (per timestep t, reset-by-subtraction, threshold 1):
    cur1 = x_t @ w1.T + b1
    m1   = beta*m1 + cur1 - (m1_prev > 1)
    spk1 = (m1 > 1)
    cur2 = spk1 @ w2.T + b2
    m2   = beta*m2 + cur2 - (m2_prev > 1)
    spk2 = (m2 > 1)
Outputs: (spk2_rec, mem2_rec), each [T, B, O].

Kernel formulation (shifted s = m - 1, sign-carry v = 2*beta*s - sigma1,
sigma1 = 2*spike in {0,2}; layer 2 uses sigma2 = sign(s2) in {-1,+1}):
    s_t     = 0.5*v_{t-1} + c'_t          (c' folds bias + beta offsets)
    v_t     = 2*beta*s_t - sigma_t
Layer 2 consumes sigma1 directly: spk1 = sigma1/2, so w2 is pre-halved
and b2 folded via a K=2 ones matmul.

The kernel is HBM-bandwidth bound at 8 cores, so x ships at 3 bytes per
element: xh = fp16(x) (2B, scaled 2^6) and f8(xl*2^12) (1B e4m3 of the
fp16 residual). The fp8 image of xh needed for the w-residual term is
cast ON DEVICE by the (otherwise idle) Act engine with the 2^-6 descale
folded into the Copy. mm1 = two PE passes into one PSUM group at 2^12:
    (xh*2^6) @ (wh*2^6)                      fp16, 7 k-chunks
    [f8(xl*2^12); f8(x)]_c @ [f8(wh); f8(wl*2^12)]_c   fp8 DR, 7 chunks
(per 128-row chunk c the DR pair computes xl_c@wh_c + xh_c@wl_c).
The 2^12 descale rides free in the scan's scalar slots: S = 2048*v +
PSUM (= 4096*s); sign tests are scale-invariant; v = (2beta/4096)*S - sigma.

x is pre-transposed AND pre-blocked on the host so each block needs two
contiguous DMAs ([128, 6*M] + remainder rows), no padding overhead.

The per-block work is software-pipelined on BOTH serial engines: mm1 of
block n+1 is emitted BEFORE the scan/mm2 of block n (the in-order PE
queue computes the next block's matmuls while the serial DVE scan runs),
and the layer-2 scan runs one block BEHIND (its cur2 was staged to SBUF
last block), so the DVE never waits for this block's mm2->Act round-trip.
Together these took 644us -> 193us. Each PSUM region's accumulation
group must stay contiguous (open+close per hc); interleaving the four hc
groups of a bank corrupts results.

Sharding: data-parallel over batch, B=512 -> 64 per core on 8 cores.
"""

import contextlib

import numpy as np
import ml_dtypes

import concourse.bacc as bacc
import concourse.mybir as mybir
from concourse import tile

BETA = 0.95
T, B, I, H, O = 100, 512, 784, 512, 10
NCORES = 8
BL = B // NCORES          # 64 batch per core
TBLK = 4                  # timesteps per block
NB = T // TBLK            # 25 blocks
M = TBLK * BL             # 256 moving columns per block
KA = I + 1                # 785 contraction (ones row appended)
NKC = 6                   # full 128-row k-chunks (fp16 passes)
KREM = KA - NKC * 128     # 17 (includes the ones row)
NDC = 7                   # paired fp8 DR chunks ([xl_c; xh8_c] vs [wh_c; wl_c])
NHC = H // 128            # 4 h-chunks
SC = 4096.0               # mm1 PSUM scale (2^12)

F32 = mybir.dt.float32
F16 = mybir.dt.float16
F8 = mybir.dt.float8e4
AL = mybir.AluOpType
AF = mybir.ActivationFunctionType
EPS = 1e-20


def build_nc(reps: int = 1, mode: str = "full"):
    """reps > 1 wraps the whole pipeline in a hardware loop (for timing).
    mode: 'full' | 'mm' (matmuls+DMA only) | 'scan' (scan chain only)."""
    nc = bacc.Bacc("TRN2", target_bir_lowering=False, debug=False)

    xhm = nc.dram_tensor("xhm", [NB, 128, NKC * M], F16, kind="ExternalInput")
    xhr = nc.dram_tensor("xhr", [NB, KREM, M], F16, kind="ExternalInput")
    x8m = nc.dram_tensor("x8m", [NB, 128, 6 * M], F8, kind="ExternalInput")
    x8r = nc.dram_tensor("x8r", [NB, KREM, M], F8, kind="ExternalInput")
    w1h = nc.dram_tensor("w1h", [128, 7 * H], F16, kind="ExternalInput")
    w18 = nc.dram_tensor("w18", [128, NDC * 2 * H], F8, kind="ExternalInput")
    w2hl = nc.dram_tensor("w2hl", [2 * H, O], F16, kind="ExternalInput")
    b2hl = nc.dram_tensor("b2hl", [2, O], F16, kind="ExternalInput")
    spk2o = nc.dram_tensor("spk2", [O, T * BL], F8, kind="ExternalOutput")
    mem2o = nc.dram_tensor("mem2", [O, T * BL], F16, kind="ExternalOutput")

    with tile.TileContext(nc) as tc:
        with (
            tc.tile_pool(name="const", bufs=1) as cpool,
            tc.tile_pool(name="xhin", bufs=3) as xhpool,
            tc.tile_pool(name="x8in", bufs=3) as x8pool,
            tc.tile_pool(name="s1p", bufs=3) as s1pool,
            tc.tile_pool(name="sg1p", bufs=2) as sg1pool,
            tc.tile_pool(name="ps1p", bufs=2, space="PSUM") as ps1pool,
            tc.tile_pool(name="ps2p", bufs=2, space="PSUM") as ps2pool,
        ):
            # resident weights (pre-padded/blocked on host; one DMA each)
            w1hs = cpool.tile([128, 7 * H], F16)
            nc.sync.dma_start(out=w1hs[:, :], in_=w1h.ap()[:, :])
            w18s = cpool.tile([128, NDC * 2 * H], F8)
            nc.sync.dma_start(out=w18s[:, :], in_=w18.ap()[:, :])
            # w2 hi/lo: [128, (pair, hc, O)]
            w2s = cpool.tile([128, 2 * NHC * O], F16)
            for p in range(2):
                for hc in range(NHC):
                    nc.sync.dma_start(
                        out=w2s[:, (p * NHC + hc) * O:(p * NHC + hc + 1) * O],
                        in_=w2hl.ap()[p * H + hc * 128:p * H + (hc + 1) * 128, :],
                    )
            b2s = cpool.tile([2, O], F16)
            nc.sync.dma_start(out=b2s[:, :], in_=b2hl.ap()[:, :])

            onesb = cpool.tile([2, M], F16)
            nc.vector.memset(onesb[:, :], 1.0)
            biasb = cpool.tile([128, 1], F32)
            nc.vector.memset(biasb[:, :], -EPS)

            v1 = cpool.tile([128, 256], F32)
            v2 = cpool.tile([O, BL], F32)

            s2f = cpool.tile([O, T * BL], F32)
            sg2f = cpool.tile([O, T * BL], F32)

            ps1_static = ps2_static = None
            if mode == "scan":
                ps1_static = ps1pool.tile([128, NHC * M], F32)
                nc.vector.memset(ps1_static[:, :], 0.125 * SC)
                ps2_static = ps2pool.tile([128, M], F32)
                nc.vector.memset(ps2_static[:, :], 0.125)

            rep_ctx = tc.For_i(0, reps, 1) if reps > 1 else contextlib.nullcontext()
            with rep_ctx:
                nc.vector.memset(v1[:, :], -2.0 * BETA)
                nc.vector.memset(v2[:, :], 1.0 - 2.0 * BETA)
                _body(nc, tc, locals(), mode)

            # ---- finalize: spk2 = 0.5*sigma2 + 0.5 ; mem2 = s2 + 1 ----
            spk8 = cpool.tile([O, T * BL], F8)
            mem16 = cpool.tile([O, T * BL], F16)
            nc.vector.tensor_scalar(
                out=spk8[:, :], in0=sg2f[:, :], scalar1=0.5, scalar2=0.5,
                op0=AL.mult, op1=AL.add,
            )
            nc.vector.tensor_scalar(
                out=mem16[:, :], in0=s2f[:, :], scalar1=1.0, scalar2=None,
                op0=AL.add,
            )
            nc.sync.dma_start(out=spk2o.ap()[:, :], in_=spk8[:, :])
            nc.sync.dma_start(out=mem2o.ap()[:, :], in_=mem16[:, :])

    nc.compile()
    return nc


def _body(nc, tc, env, mode="full"):
    (xhm, xhr, x8m, x8r, w1hs, w18s, w2s, b2s, onesb, biasb,
     v1, v2, s2f, sg2f,
     xhpool, x8pool, s1pool, sg1pool, ps1pool, ps2pool) = (
        env["xhm"], env["xhr"], env["x8m"], env["x8r"], env["w1hs"],
        env["w18s"], env["w2s"], env["b2s"],
        env["onesb"], env["biasb"], env["v1"], env["v2"], env["s2f"],
        env["sg2f"], env["xhpool"], env["x8pool"], env["s1pool"],
        env["sg1pool"], env["ps1pool"], env["ps2pool"],
    )
    ps1_static, ps2_static = env["ps1_static"], env["ps2_static"]
    do_mm = mode in ("full", "mm", "mm1")
    do_mm2 = mode in ("full", "mm")
    do_scan = mode in ("full", "scan")

    def load_and_mm1(blk):
        """DMA x block, Act-cast xh->fp8 (descale 2^-6), run mm1 into a
        fresh PSUM tile. Emitted one block AHEAD of the scan so the PE
        stays busy while the (serial) scan of the previous block runs."""
        xbh = xhpool.tile([128, 7 * M], F16, tag="xbh")
        xb8 = x8pool.tile([128, NDC * 2 * M], F8, tag="xb8")
        nc.sync.dma_start(out=xbh[:, 0:NKC * M], in_=xhm.ap()[blk, :, :])
        nc.sync.dma_start(
            out=xbh[0:KREM, NKC * M:7 * M], in_=xhr.ap()[blk, :, :]
        )
        xb8v = xb8.rearrange("p (c i m) -> p c i m", c=NDC, i=2)
        nc.sync.dma_start(
            out=xb8v[:, 0:6, 0, :],
            in_=x8m.ap()[blk, :, :].rearrange("p (c m) -> p c m", c=6),
        )
        nc.sync.dma_start(
            out=xb8v[0:KREM, 6, 0, :], in_=x8r.ap()[blk, :, :]
        )
        nc.scalar.activation(
            out=xb8v[:, :, 1, :],
            in_=xbh.rearrange("p (c m) -> p c m", c=7),
            func=AF.Copy, scale=1.0 / 64.0,
        )
        ps1 = ps1pool.tile([128, NHC * M], F32, tag="ps1")
        for hc in range(NHC):
            for kc in range(7):
                kp = 128 if kc < NKC else KREM
                nc.tensor.matmul(
                    ps1[:, hc * M:(hc + 1) * M],
                    lhsT=w1hs[0:kp, kc * H + hc * 128:kc * H + hc * 128 + 128],
                    rhs=xbh[0:kp, kc * M:(kc + 1) * M],
                    start=(kc == 0),
                    stop=False,
                )
            for dc in range(NDC):
                dp = 128 if dc < 6 else KREM
                lt = w18s[:, dc * 2 * H:(dc + 1) * 2 * H].rearrange(
                    "p (i h) -> p i h", i=2
                )
                rt = xb8[:, dc * 2 * M:(dc + 1) * 2 * M].rearrange(
                    "p (i m) -> p i m", i=2
                )
                nc.tensor.matmul(
                    ps1[:, hc * M:(hc + 1) * M],
                    lhsT=lt[0:dp, :, hc * 128:hc * 128 + 128],
                    rhs=rt[0:dp, :, :],
                    start=False,
                    stop=(dc == NDC - 1),
                    perf_mode=mybir.MatmulPerfMode.DoubleRow,
                )
        return ps1

    pend_l2 = [None]
    ps1_next = load_and_mm1(0) if do_mm else None
    for blk in range(NB):
        ps1 = ps1_next if do_mm else ps1_static
        if do_mm and blk + 1 < NB:
            ps1_next = load_and_mm1(blk + 1)

        # ---- layer-1 scan over the TBLK timesteps (S = 4096*s) ----
        if do_scan:
            sg1 = sg1pool.tile([128, NHC * M], F16)
        else:
            sg1 = None
        ps1v = ps1.rearrange("p (c t b) -> p c t b", c=NHC, t=TBLK)
        sg1v = (sg1.rearrange("p (c t b) -> p c t b", c=NHC, t=TBLK)
                if sg1 is not None else None)
        v1v = v1.rearrange("p (c b) -> p c b", c=NHC)
        for t in range(TBLK) if do_scan else ():
            s1 = s1pool.tile([128, 256], F32)
            s1v = s1.rearrange("p (c b) -> p c b", c=NHC)
            nc.vector.scalar_tensor_tensor(
                out=s1v, in0=v1v, scalar=SC / 2.0, in1=ps1v[:, :, t, :],
                op0=AL.mult, op1=AL.add,
            )
            # sigma1 = (S > 0) * 2 in {0,2}; exact strict-> test, fp16 out
            nc.vector.tensor_scalar(
                out=sg1v[:, :, t, :], in0=s1v, scalar1=0.0, scalar2=2.0,
                op0=AL.is_gt, op1=AL.mult,
            )
            nc.vector.scalar_tensor_tensor(
                out=v1v, in0=s1v, scalar=2.0 * BETA / SC, in1=sg1v[:, :, t, :],
                op0=AL.mult, op1=AL.subtract,
            )

        # ---- mm2: cur2^T [O, M] in PSUM (b2 via K=2 ones matmul) ----
        if do_mm2:
            ps2 = ps2pool.tile([128, M], F32)
        elif mode == "mm1":
            ps2 = None
        else:
            ps2 = ps2_static
        if do_mm2:
            nc.tensor.matmul(
                ps2[0:O, :],
                lhsT=b2s[0:2, :],
                rhs=onesb[0:2, :],
                start=True, stop=False,
            )
        for p in range(2) if do_mm2 else ():
            for hc in range(NHC):
                nc.tensor.matmul(
                    ps2[0:O, :],
                    lhsT=w2s[:, (p * NHC + hc) * O:(p * NHC + hc + 1) * O],
                    rhs=(sg1 if sg1 is not None else w1hs)[
                        :, hc * M:(hc + 1) * M],
                    start=False, stop=(p == 1 and hc == NHC - 1),
                )

        # ---- layer-2 scan, pipelined ONE BLOCK BEHIND so the DVE never
        #      waits for this block's mm2->Act staging round-trip ----
        if do_scan and ps2 is not None:
            if pend_l2[0] is not None:
                _scan_l2(nc, env, *pend_l2[0])
            c2s = s1pool.tile([O, M], F32, tag="c2s")
            nc.scalar.activation(
                out=c2s[:, :], in_=ps2[0:O, :], func=AF.Copy,
            )
            pend_l2[0] = (blk, c2s)
    if do_scan and pend_l2[0] is not None:
        _scan_l2(nc, env, *pend_l2[0])


def _scan_l2(nc, env, blk, c2s):
    v2, s2f, sg2f, biasb = env["v2"], env["s2f"], env["sg2f"], env["biasb"]
    for t in range(TBLK):
        g0 = (blk * TBLK + t) * BL
        s2sl = s2f[:, g0:g0 + BL]
        sg2sl = sg2f[:, g0:g0 + BL]
        nc.vector.scalar_tensor_tensor(
            out=s2sl, in0=v2[:, :], scalar=0.5,
            in1=c2s[:, t * BL:(t + 1) * BL],
            op0=AL.mult, op1=AL.add,
        )
        nc.scalar.activation(
            out=sg2sl, in_=s2sl, func=AF.Sign,
            bias=biasb[0:O, 0:1], scale=1.0,
        )
        nc.vector.scalar_tensor_tensor(
            out=v2[:, :], in0=s2sl, scalar=2.0 * BETA, in1=sg2sl,
            op0=AL.mult, op1=AL.subtract,
        )


def _f8(a):
    return a.astype(ml_dtypes.float8_e4m3)


def make_in_maps(x, w1, b1, w2, b2):
    """Host-side sharding + layout marshaling."""
    x = np.asarray(x, dtype=np.float32)
    w1 = np.asarray(w1, dtype=np.float32)
    b1 = np.asarray(b1, dtype=np.float32)
    w2 = np.asarray(w2, dtype=np.float32)
    b2 = np.asarray(b2, dtype=np.float32)

    w1T_aug = np.empty((KA, H), dtype=np.float32)
    w1T_aug[:I] = w1.T
    w1T_aug[I] = b1 + (BETA - 1.0)
    wh16 = w1T_aug.astype(np.float16)                   # unscaled fp16 hi
    wl = w1T_aug - wh16.astype(np.float32)

    def pack_kchunks(rows, dtype):                      # [KA-ish, H] -> [128, 7H]
        out = np.zeros((128, 7, H), dtype=dtype)
        K_ = rows.shape[0]
        for c in range(7):
            n = min(128, K_ - c * 128)
            if n > 0:
                out[:n, c, :] = rows[c * 128:c * 128 + n]
        return out.reshape(128, 7 * H)

    w1h_in = pack_kchunks((wh16.astype(np.float32) * 64.0).astype(np.float16),
                          np.float16)

    # paired fp8 DR weights: chunk c = [f8(wh_c) ; f8(wl_c * SC)]
    w8hi = _f8(wh16.astype(np.float32))                 # [785, H]
    w8lo = _f8(wl * SC)                                 # [785, H]
    w18_in = np.zeros((128, NDC, 2, H), dtype=ml_dtypes.float8_e4m3)
    for dc in range(NDC):
        n = min(128, KA - dc * 128)
        w18_in[:n, dc, 0, :] = w8hi[dc * 128:dc * 128 + n]
        w18_in[:n, dc, 1, :] = w8lo[dc * 128:dc * 128 + n]
    w18_in = w18_in.reshape(128, NDC * 2 * H)

    w2T_half = np.ascontiguousarray(w2.T) * 0.5
    w2h = w2T_half.astype(np.float16)
    w2l = (w2T_half - w2h.astype(np.float32)).astype(np.float16)
    w2hl = np.concatenate([w2h, w2l], axis=0)           # [2H, O]

    b2_aug = (b2 + (BETA - 1.5)).astype(np.float32)
    b2h = b2_aug.reshape(1, O).astype(np.float16)
    b2l = (b2_aug.reshape(1, O) - b2h.astype(np.float32)).astype(np.float16)
    b2hl = np.concatenate([b2h, b2l], axis=0)           # [2, O]

    in_maps = []
    for c in range(NCORES):
        xc = x[:, c * BL:(c + 1) * BL, :]               # [T, BL, I]
        xTc = np.empty((KA, T * BL), dtype=np.float32)
        xTc[:I] = xc.reshape(T * BL, I).T
        xTc[I] = 1.0
        xh16 = xTc.astype(np.float16)                   # unscaled fp16 hi
        xl = xTc - xh16.astype(np.float32)              # residual (row I: 0)
        xh_sc = (xh16.astype(np.float32) * 64.0).astype(np.float16)
        xl8 = _f8(xl[:I] * SC)                          # [784, M-cols]

        xh_b = xh_sc.reshape(KA, NB, M)
        xhm_in = np.ascontiguousarray(
            xh_b[:NKC * 128].reshape(NKC, 128, NB, M).transpose(2, 1, 0, 3)
        ).reshape(NB, 128, NKC * M)
        xhr_in = np.ascontiguousarray(
            xh_b[NKC * 128:].transpose(1, 0, 2)
        )                                               # [NB, 17, M]

        x8_b = xl8.reshape(I, NB, M)
        x8m_in = np.ascontiguousarray(
            x8_b[:6 * 128].reshape(6, 128, NB, M).transpose(2, 1, 0, 3)
        ).reshape(NB, 128, 6 * M)
        x8r_in = np.zeros((NB, KREM, M), dtype=ml_dtypes.float8_e4m3)
        x8r_in[:, :I - 6 * 128, :] = x8_b[6 * 128:].transpose(1, 0, 2)

        in_maps.append({
            "xhm": xhm_in, "xhr": xhr_in, "x8m": x8m_in, "x8r": x8r_in,
            "w1h": w1h_in, "w18": w18_in,
            "w2hl": w2hl, "b2hl": b2hl,
        })
    return in_maps


def gather_outputs(results):
    """results: list of per-core {'spk2': [O, T*BL] f8, 'mem2': [O, T*BL] f16}."""
    spk = np.empty((T, B, O), dtype=np.float32)
    mem = np.empty((T, B, O), dtype=np.float32)
    for c, r in enumerate(results):
        spk[:, c * BL:(c + 1) * BL, :] = (
            r["spk2"].astype(np.float32).reshape(O, T, BL).transpose(1, 2, 0)
        )
        mem[:, c * BL:(c + 1) * BL, :] = (
            r["mem2"].astype(np.float32).reshape(O, T, BL).transpose(1, 2, 0)
        )
    return spk, mem


_NC_CACHE = None


def kernel(x, w1, b1, w2, b2):
    global _NC_CACHE
    from concourse import bass_utils

    if _NC_CACHE is None:
        _NC_CACHE = build_nc()
    in_maps = make_in_maps(x, w1, b1, w2, b2)
    res = bass_utils.run_bass_kernel_spmd(
        _NC_CACHE, in_maps, core_ids=list(range(NCORES))
    )
    return gather_outputs(res.results)



# revision 5
# speedup vs baseline: 1.0923x; 1.0923x over previous
"""Trainium2 Bass kernel for a 2-layer spiking (LIF) network.

Reference semantics (per timestep t, reset-by-subtraction, threshold 1):
    cur1 = x_t @ w1.T + b1
    m1   = beta*m1 + cur1 - (m1_prev > 1)
    spk1 = (m1 > 1)
    cur2 = spk1 @ w2.T + b2
    m2   = beta*m2 + cur2 - (m2_prev > 1)
    spk2 = (m2 > 1)
Outputs: (spk2_rec, mem2_rec), each [T, B, O].

Kernel formulation (shifted s = m - 1, sign-carry v = 2*beta*s - sigma1,
sigma1 = 2*spike in {0,2}; layer 2 uses sigma2 = sign(s2) in {-1,+1}):
    s_t     = 0.5*v_{t-1} + c'_t          (c' folds bias + beta offsets)
    v_t     = 2*beta*s_t - sigma_t
Layer 2 consumes sigma1 directly: spk1 = sigma1/2, so w2 is pre-halved;
b2 (+ beta-1.5) is folded into the per-partition bias of the Act-engine
Copy that stages cur2 from PSUM to SBUF (no bias matmul pass).

The kernel is PE-pass bound (65 passes/block took 193us; DMA and the
DVE scan have ~2-3x headroom), so passes are pared to the accuracy
budget (harness gate: rel < 2e-2; measured ~1.35e-2 for this config):
    mm1 = xh16 @ wh16  +  xh8 @ wl8          (x residual DROPPED)
where xh = fp16(x) (2B/elem, scaled 2^6), wh = fp16(w1), wl8 =
f8((w1 - wh) * 2^12). The fp8 image xh8 of xh is cast ON DEVICE by the
(otherwise idle) Act engine with the 2^-6 descale folded into the Copy.
Per 128-col h-chunk: 7 fp16 k-passes + 3 fp8 DoubleRow passes (adjacent
k-chunk pairs 0-5) + 1 plain fp8 pass (17-row k-remainder) = 11 passes,
all one PSUM accumulation group at scale 2^12. The 2^12 descale rides
free in the scan's scalar slots: S = 2048*v + PSUM (= 4096*s); sign
tests are scale-invariant; v = (2beta/4096)*S - sigma.
mm2 = sigma1 @ (w2.T/2) in single fp16: 4 passes. 48 passes/block total.

x is pre-transposed AND pre-blocked on the host so each block needs two
contiguous DMAs ([128, 6*M] + remainder rows), no padding overhead.

The per-block work is software-pipelined on BOTH serial engines: mm1 of
block n+1 is emitted BEFORE the scan/mm2 of block n (the in-order PE
queue computes the next block's matmuls while the serial DVE scan runs),
and the layer-2 scan runs one block BEHIND (its cur2 was staged to SBUF
last block), so the DVE never waits for this block's mm2->Act round-trip.
Each PSUM region's accumulation group must stay contiguous (open+close
per hc); interleaving the four hc groups of a bank corrupts results.

Sharding: data-parallel over batch, B=512 -> 64 per core on 8 cores.
"""

import contextlib

import numpy as np
import ml_dtypes

import concourse.bacc as bacc
import concourse.mybir as mybir
from concourse import tile

BETA = 0.95
T, B, I, H, O = 100, 512, 784, 512, 10
NCORES = 8
BL = B // NCORES          # 64 batch per core
TBLK = 4                  # timesteps per block
NB = T // TBLK            # 25 blocks
M = TBLK * BL             # 256 moving columns per block
KA = I + 1                # 785 contraction (ones row appended)
NKC = 6                   # full 128-row k-chunks (fp16 passes)
KREM = KA - NKC * 128     # 17 (includes the ones row)
NHC = H // 128            # 4 h-chunks
SC = 4096.0               # mm1 PSUM scale (2^12)

F32 = mybir.dt.float32
F16 = mybir.dt.float16
F8 = mybir.dt.float8e4
AL = mybir.AluOpType
AF = mybir.ActivationFunctionType
EPS = 1e-20


def build_nc(reps: int = 1, mode: str = "full"):
    """reps > 1 wraps the whole pipeline in a hardware loop (for timing).
    mode: 'full' | 'mm' (matmuls+DMA only) | 'scan' (scan chain only)."""
    nc = bacc.Bacc("TRN2", target_bir_lowering=False, debug=False)

    xhm = nc.dram_tensor("xhm", [NB, 128, NKC * M], F16, kind="ExternalInput")
    xhr = nc.dram_tensor("xhr", [NB, KREM, M], F16, kind="ExternalInput")
    w1h = nc.dram_tensor("w1h", [128, 7 * H], F16, kind="ExternalInput")
    w18 = nc.dram_tensor("w18", [128, 7 * H], F8, kind="ExternalInput")
    w2h = nc.dram_tensor("w2h", [H, O], F16, kind="ExternalInput")
    b2c = nc.dram_tensor("b2c", [O, 1], F32, kind="ExternalInput")
    spk2o = nc.dram_tensor("spk2", [O, T * BL], F8, kind="ExternalOutput")
    mem2o = nc.dram_tensor("mem2", [O, T * BL], F16, kind="ExternalOutput")

    with tile.TileContext(nc) as tc:
        with (
            tc.tile_pool(name="const", bufs=1) as cpool,
            tc.tile_pool(name="xhin", bufs=3) as xhpool,
            tc.tile_pool(name="x8in", bufs=3) as x8pool,
            tc.tile_pool(name="s1p", bufs=3) as s1pool,
            tc.tile_pool(name="sg1p", bufs=2) as sg1pool,
            tc.tile_pool(name="ps1p", bufs=2, space="PSUM") as ps1pool,
            tc.tile_pool(name="ps2p", bufs=2, space="PSUM") as ps2pool,
        ):
            # resident weights (pre-padded/blocked on host; one DMA each)
            w1hs = cpool.tile([128, 7 * H], F16)
            nc.sync.dma_start(out=w1hs[:, :], in_=w1h.ap()[:, :])
            w18s = cpool.tile([128, 7 * H], F8)
            nc.sync.dma_start(out=w18s[:, :], in_=w18.ap()[:, :])
            # w2 (pre-halved fp16): [128, (hc, O)]
            w2s = cpool.tile([128, NHC * O], F16)
            for hc in range(NHC):
                nc.sync.dma_start(
                    out=w2s[:, hc * O:(hc + 1) * O],
                    in_=w2h.ap()[hc * 128:(hc + 1) * 128, :],
                )
            b2s = cpool.tile([O, 1], F32)
            nc.sync.dma_start(out=b2s[:, :], in_=b2c.ap()[:, :])

            biasb = cpool.tile([128, 1], F32)
            nc.vector.memset(biasb[:, :], -EPS)

            v1 = cpool.tile([128, 256], F32)
            v2 = cpool.tile([O, BL], F32)

            s2f = cpool.tile([O, T * BL], F32)
            sg2f = cpool.tile([O, T * BL], F32)

            ps1_static = ps2_static = None
            if mode == "scan":
                ps1_static = ps1pool.tile([128, NHC * M], F32)
                nc.vector.memset(ps1_static[:, :], 0.125 * SC)
                ps2_static = ps2pool.tile([128, M], F32)
                nc.vector.memset(ps2_static[:, :], 0.125)

            rep_ctx = tc.For_i(0, reps, 1) if reps > 1 else contextlib.nullcontext()
            with rep_ctx:
                nc.vector.memset(v1[:, :], -2.0 * BETA)
                nc.vector.memset(v2[:, :], 1.0 - 2.0 * BETA)
                _body(nc, tc, locals(), mode)

            # ---- finalize: spk2 = 0.5*sigma2 + 0.5 ; mem2 = s2 + 1 ----
            spk8 = cpool.tile([O, T * BL], F8)
            mem16 = cpool.tile([O, T * BL], F16)
            nc.vector.tensor_scalar(
                out=spk8[:, :], in0=sg2f[:, :], scalar1=0.5, scalar2=0.5,
                op0=AL.mult, op1=AL.add,
            )
            nc.vector.tensor_scalar(
                out=mem16[:, :], in0=s2f[:, :], scalar1=1.0, scalar2=None,
                op0=AL.add,
            )
            nc.sync.dma_start(out=spk2o.ap()[:, :], in_=spk8[:, :])
            nc.sync.dma_start(out=mem2o.ap()[:, :], in_=mem16[:, :])

    nc.compile()
    return nc


def _body(nc, tc, env, mode="full"):
    (xhm, xhr, w1hs, w18s, w2s, b2s, biasb,
     v1, v2, s2f, sg2f,
     xhpool, x8pool, s1pool, sg1pool, ps1pool, ps2pool) = (
        env["xhm"], env["xhr"], env["w1hs"],
        env["w18s"], env["w2s"], env["b2s"],
        env["biasb"], env["v1"], env["v2"], env["s2f"],
        env["sg2f"], env["xhpool"], env["x8pool"], env["s1pool"],
        env["sg1pool"], env["ps1pool"], env["ps2pool"],
    )
    ps1_static, ps2_static = env["ps1_static"], env["ps2_static"]
    do_mm = mode in ("full", "mm", "mm1")
    do_mm2 = mode in ("full", "mm")
    do_scan = mode in ("full", "scan")

    def load_and_mm1(blk):
        """DMA x block, Act-cast xh->fp8 (descale 2^-6), run mm1 into a
        fresh PSUM tile. Emitted one block AHEAD of the scan so the PE
        stays busy while the (serial) scan of the previous block runs."""
        xbh = xhpool.tile([128, 7 * M], F16, tag="xbh")
        xb8 = x8pool.tile([128, 7 * M], F8, tag="xb8")
        nc.sync.dma_start(out=xbh[:, 0:NKC * M], in_=xhm.ap()[blk, :, :])
        nc.sync.dma_start(
            out=xbh[0:KREM, NKC * M:7 * M], in_=xhr.ap()[blk, :, :]
        )
        # fp8 image of xh for the w-residual term (2^-6 descale folded in)
        nc.scalar.activation(
            out=xb8[:, :], in_=xbh[:, :], func=AF.Copy, scale=1.0 / 64.0,
        )
        ps1 = ps1pool.tile([128, NHC * M], F32, tag="ps1")
        for hc in range(NHC):
            for kc in range(7):
                kp = 128 if kc < NKC else KREM
                nc.tensor.matmul(
                    ps1[:, hc * M:(hc + 1) * M],
                    lhsT=w1hs[0:kp, kc * H + hc * 128:kc * H + hc * 128 + 128],
                    rhs=xbh[0:kp, kc * M:(kc + 1) * M],
                    start=(kc == 0),
                    stop=False,
                )
            # xh8 @ wl8: adjacent k-chunk pairs (0,1)(2,3)(4,5) as fp8
            # DoubleRow, then the 17-row k-remainder as a plain fp8 pass.
            for j in range(3):
                lt = w18s[:, 2 * j * H:(2 * j + 2) * H].rearrange(
                    "p (i h) -> p i h", i=2
                )
                rt = xb8[:, 2 * j * M:(2 * j + 2) * M].rearrange(
                    "p (i m) -> p i m", i=2
                )
                nc.tensor.matmul(
                    ps1[:, hc * M:(hc + 1) * M],
                    lhsT=lt[:, :, hc * 128:hc * 128 + 128],
                    rhs=rt[:, :, :],
                    start=False,
                    stop=False,
                    perf_mode=mybir.MatmulPerfMode.DoubleRow,
                )
            nc.tensor.matmul(
                ps1[:, hc * M:(hc + 1) * M],
                lhsT=w18s[0:KREM, 6 * H + hc * 128:6 * H + hc * 128 + 128],
                rhs=xb8[0:KREM, 6 * M:7 * M],
                start=False,
                stop=True,
            )
        return ps1

    pend_l2 = [None]
    ps1_next = load_and_mm1(0) if do_mm else None
    for blk in range(NB):
        ps1 = ps1_next if do_mm else ps1_static
        if do_mm and blk + 1 < NB:
            ps1_next = load_and_mm1(blk + 1)

        # ---- layer-1 scan over the TBLK timesteps (S = 4096*s) ----
        if do_scan:
            sg1 = sg1pool.tile([128, NHC * M], F16)
        else:
            sg1 = None
        ps1v = ps1.rearrange("p (c t b) -> p c t b", c=NHC, t=TBLK)
        sg1v = (sg1.rearrange("p (c t b) -> p c t b", c=NHC, t=TBLK)
                if sg1 is not None else None)
        v1v = v1.rearrange("p (c b) -> p c b", c=NHC)
        for t in range(TBLK) if do_scan else ():
            s1 = s1pool.tile([128, 256], F32)
            s1v = s1.rearrange("p (c b) -> p c b", c=NHC)
            nc.vector.scalar_tensor_tensor(
                out=s1v, in0=v1v, scalar=SC / 2.0, in1=ps1v[:, :, t, :],
                op0=AL.mult, op1=AL.add,
            )
            # sigma1 = (S > 0) * 2 in {0,2}; exact strict-> test, fp16 out
            nc.vector.tensor_scalar(
                out=sg1v[:, :, t, :], in0=s1v, scalar1=0.0, scalar2=2.0,
                op0=AL.is_gt, op1=AL.mult,
            )
            nc.vector.scalar_tensor_tensor(
                out=v1v, in0=s1v, scalar=2.0 * BETA / SC, in1=sg1v[:, :, t, :],
                op0=AL.mult, op1=AL.subtract,
            )

        # ---- mm2: cur2^T [O, M] in PSUM (single fp16 w2, no bias pass) ----
        if do_mm2:
            ps2 = ps2pool.tile([128, M], F32)
        elif mode == "mm1":
            ps2 = None
        else:
            ps2 = ps2_static
        for hc in range(NHC) if do_mm2 else ():
            nc.tensor.matmul(
                ps2[0:O, :],
                lhsT=w2s[:, hc * O:(hc + 1) * O],
                rhs=(sg1 if sg1 is not None else w1hs)[
                    :, hc * M:(hc + 1) * M],
                start=(hc == 0), stop=(hc == NHC - 1),
            )

        # ---- layer-2 scan, pipelined ONE BLOCK BEHIND so the DVE never
        #      waits for this block's mm2->Act staging round-trip ----
        if do_scan and ps2 is not None:
            if pend_l2[0] is not None:
                _scan_l2(nc, env, *pend_l2[0])
            c2s = s1pool.tile([O, M], F32, tag="c2s")
            # b2 + (beta - 1.5) rides the Act Identity's per-partition bias
            nc.scalar.activation(
                out=c2s[:, :], in_=ps2[0:O, :], func=AF.Identity,
                bias=b2s[0:O, 0:1], scale=1.0,
            )
            pend_l2[0] = (blk, c2s)
    if do_scan and pend_l2[0] is not None:
        _scan_l2(nc, env, *pend_l2[0])


def _scan_l2(nc, env, blk, c2s):
    v2, s2f, sg2f, biasb = env["v2"], env["s2f"], env["sg2f"], env["biasb"]
    for t in range(TBLK):
        g0 = (blk * TBLK + t) * BL
        s2sl = s2f[:, g0:g0 + BL]
        sg2sl = sg2f[:, g0:g0 + BL]
        nc.vector.scalar_tensor_tensor(
            out=s2sl, in0=v2[:, :], scalar=0.5,
            in1=c2s[:, t * BL:(t + 1) * BL],
            op0=AL.mult, op1=AL.add,
        )
        nc.scalar.activation(
            out=sg2sl, in_=s2sl, func=AF.Sign,
            bias=biasb[0:O, 0:1], scale=1.0,
        )
        nc.vector.scalar_tensor_tensor(
            out=v2[:, :], in0=s2sl, scalar=2.0 * BETA, in1=sg2sl,
            op0=AL.mult, op1=AL.subtract,
        )


def _f8(a):
    return a.astype(ml_dtypes.float8_e4m3)


def make_in_maps(x, w1, b1, w2, b2):
    """Host-side sharding + layout marshaling."""
    x = np.asarray(x, dtype=np.float32)
    w1 = np.asarray(w1, dtype=np.float32)
    b1 = np.asarray(b1, dtype=np.float32)
    w2 = np.asarray(w2, dtype=np.float32)
    b2 = np.asarray(b2, dtype=np.float32)

    w1T_aug = np.empty((KA, H), dtype=np.float32)
    w1T_aug[:I] = w1.T
    # x ships centered (u = x - 0.5): halves the fp16 ulp of the x-hi
    # stream; 0.5*colsum(w1) moves into the ones-row bias.
    w1T_aug[I] = b1 + (BETA - 1.0) + 0.5 * w1.T.sum(axis=0)
    wh16 = w1T_aug.astype(np.float16)                   # unscaled fp16 hi
    wl = w1T_aug - wh16.astype(np.float32)

    def pack_kchunks(rows, dtype):                      # [KA-ish, H] -> [128, 7H]
        out = np.zeros((128, 7, H), dtype=dtype)
        K_ = rows.shape[0]
        for c in range(7):
            n = min(128, K_ - c * 128)
            if n > 0:
                out[:n, c, :] = rows[c * 128:c * 128 + n]
        return out.reshape(128, 7 * H)

    w1h_in = pack_kchunks((wh16.astype(np.float32) * 64.0).astype(np.float16),
                          np.float16)
    w18_in = pack_kchunks(_f8(wl * SC), ml_dtypes.float8_e4m3)

    w2h_in = (np.ascontiguousarray(w2.T) * 0.5).astype(np.float16)  # [H, O]
    b2c_in = (b2 + (BETA - 1.5)).astype(np.float32).reshape(O, 1)

    in_maps = []
    for c in range(NCORES):
        xc = x[:, c * BL:(c + 1) * BL, :]               # [T, BL, I]
        xTc = np.empty((KA, T * BL), dtype=np.float32)
        xTc[:I] = xc.reshape(T * BL, I).T
        xTc[:I] -= 0.5                                  # centered (see above)
        xTc[I] = 1.0
        xh_sc = (xTc.astype(np.float16).astype(np.float32)
                 * 64.0).astype(np.float16)

        xh_b = xh_sc.reshape(KA, NB, M)
        xhm_in = np.ascontiguousarray(
            xh_b[:NKC * 128].reshape(NKC, 128, NB, M).transpose(2, 1, 0, 3)
        ).reshape(NB, 128, NKC * M)
        xhr_in = np.ascontiguousarray(
            xh_b[NKC * 128:].transpose(1, 0, 2)
        )                                               # [NB, 17, M]

        in_maps.append({
            "xhm": xhm_in, "xhr": xhr_in,
            "w1h": w1h_in, "w18": w18_in,
            "w2h": w2h_in, "b2c": b2c_in,
        })
    return in_maps


def gather_outputs(results):
    """results: list of per-core {'spk2': [O, T*BL] f8, 'mem2': [O, T*BL] f16}."""
    spk = np.empty((T, B, O), dtype=np.float32)
    mem = np.empty((T, B, O), dtype=np.float32)
    for c, r in enumerate(results):
        spk[:, c * BL:(c + 1) * BL, :] = (
            r["spk2"].astype(np.float32).reshape(O, T, BL).transpose(1, 2, 0)
        )
        mem[:, c * BL:(c + 1) * BL, :] = (
            r["mem2"].astype(np.float32).reshape(O, T, BL).transpose(1, 2, 0)
        )
    return spk, mem


_NC_CACHE = None


def kernel(x, w1, b1, w2, b2):
    global _NC_CACHE
    from concourse import bass_utils

    if _NC_CACHE is None:
        _NC_CACHE = build_nc()
    in_maps = make_in_maps(x, w1, b1, w2, b2)
    res = bass_utils.run_bass_kernel_spmd(
        _NC_CACHE, in_maps, core_ids=list(range(NCORES))
    )
    return gather_outputs(res.results)


# revision 7
# speedup vs baseline: 3.1793x; 2.9107x over previous
"""Trainium2 Bass kernel for a 2-layer spiking (LIF) network.

Reference semantics (per timestep t, reset-by-subtraction, threshold 1):
    cur1 = x_t @ w1.T + b1
    m1   = beta*m1 + cur1 - (m1_prev > 1)
    spk1 = (m1 > 1)
    cur2 = spk1 @ w2.T + b2
    m2   = beta*m2 + cur2 - (m2_prev > 1)
    spk2 = (m2 > 1)
Outputs: (spk2_rec, mem2_rec), each [T, B, O].

Kernel formulation (shifted s = m - 1, sign-carry v = 2*beta*s - sigma1,
sigma1 = 2*spike in {0,2}; layer 2 uses sigma2 = sign(s2) in {-1,+1}):
    s_t     = 0.5*v_{t-1} + c'_t          (c' folds bias + beta offsets)
    v_t     = 2*beta*s_t - sigma_t
Layer 2 consumes sigma1 directly: spk1 = sigma1/2, so w2 is pre-halved;
b2 (+ beta-1.5) is folded into the per-partition bias of the Act-engine
Copy that stages cur2 from PSUM to SBUF (no bias matmul pass).

The kernel is PE-pass bound (65 passes/block took 193us; DMA and the
DVE scan have ~2-3x headroom), so passes are pared to the accuracy
budget (harness gate: rel < 2e-2; measured ~1.35e-2 for this config):
    mm1 = xh16 @ wh16  +  xh8 @ wl8          (x residual DROPPED)
where xh = fp16(x) (2B/elem, scaled 2^6), wh = fp16(w1), wl8 =
f8((w1 - wh) * 2^12). The fp8 image xh8 of xh is cast ON DEVICE by the
(otherwise idle) Act engine with the 2^-6 descale folded into the Copy.
Per 128-col h-chunk: 7 fp16 k-passes + 3 fp8 DoubleRow passes (adjacent
k-chunk pairs 0-5) + 1 plain fp8 pass (17-row k-remainder) = 11 passes,
all one PSUM accumulation group at scale 2^12. The 2^12 descale rides
free in the scan's scalar slots: S = 2048*v + PSUM (= 4096*s); sign
tests are scale-invariant; v = (2beta/4096)*S - sigma.
mm2 = sigma1 @ (w2.T/2) in single fp16: 4 passes. 48 passes/block total.

x is pre-transposed AND pre-blocked on the host so each block needs two
contiguous DMAs ([128, 6*M] + remainder rows), no padding overhead.

The per-block work is software-pipelined on BOTH serial engines: mm1 of
block n+1 is emitted BEFORE the scan/mm2 of block n (the in-order PE
queue computes the next block's matmuls while the serial DVE scan runs),
and the layer-2 scan runs one block BEHIND (its cur2 was staged to SBUF
last block), so the DVE never waits for this block's mm2->Act round-trip.
Each PSUM region's accumulation group must stay contiguous (open+close
per hc); interleaving the four hc groups of a bank corrupts results.

Sharding: data-parallel over batch, B=512 -> 64 per core on 8 cores.
"""

import contextlib

import numpy as np
import ml_dtypes

import concourse.bacc as bacc
import concourse.mybir as mybir
from concourse import tile

BETA = 0.95
T, B, I, H, O = 100, 512, 784, 512, 10
NCORES = 8
BL = B // NCORES          # 64 batch per core
TBLK = 4                  # timesteps per block
NB = T // TBLK            # 25 blocks
M = TBLK * BL             # 256 moving columns per block
KA = I + 1                # 785 contraction (ones row appended)
NKC = 6                   # full 128-row k-chunks (fp16 passes)
KREM = KA - NKC * 128     # 17 (includes the ones row)
NHC = H // 128            # 4 h-chunks
SC = 4096.0               # mm1 PSUM scale (2^12)

F32 = mybir.dt.float32
F16 = mybir.dt.float16
F8 = mybir.dt.float8e4
AL = mybir.AluOpType
AF = mybir.ActivationFunctionType
EPS = 1e-20


def build_nc(reps: int = 1, mode: str = "full"):
    """reps > 1 wraps the whole pipeline in a hardware loop (for timing).
    mode: 'full' | 'mm' (matmuls+DMA only) | 'scan' (scan chain only)."""
    nc = bacc.Bacc("TRN2", target_bir_lowering=False, debug=False)

    xhm = nc.dram_tensor("xhm", [NB, 128, NKC * M], F16, kind="ExternalInput")
    xhr = nc.dram_tensor("xhr", [NB, KREM, M], F16, kind="ExternalInput")
    w1h = nc.dram_tensor("w1h", [128, 7 * H], F16, kind="ExternalInput")
    w18 = nc.dram_tensor("w18", [128, 7 * H], F8, kind="ExternalInput")
    w2h = nc.dram_tensor("w2h", [H, O], F16, kind="ExternalInput")
    b2c = nc.dram_tensor("b2c", [O, 1], F32, kind="ExternalInput")
    spk2o = nc.dram_tensor("spk2", [O, T * BL], F8, kind="ExternalOutput")
    mem2o = nc.dram_tensor("mem2", [O, T * BL], F16, kind="ExternalOutput")

    with tile.TileContext(nc) as tc:
        with (
            tc.tile_pool(name="const", bufs=1) as cpool,
            tc.tile_pool(name="xhin", bufs=3) as xhpool,
            tc.tile_pool(name="x8in", bufs=3) as x8pool,
            tc.tile_pool(name="s1p", bufs=3) as s1pool,
            tc.tile_pool(name="sg1p", bufs=2) as sg1pool,
            tc.tile_pool(name="ps1p", bufs=2, space="PSUM") as ps1pool,
            tc.tile_pool(name="ps2p", bufs=2, space="PSUM") as ps2pool,
        ):
            # resident weights (pre-padded/blocked on host; one DMA each)
            w1hs = cpool.tile([128, 7 * H], F16)
            nc.sync.dma_start(out=w1hs[:, :], in_=w1h.ap()[:, :])
            w18s = cpool.tile([128, 7 * H], F8)
            nc.sync.dma_start(out=w18s[:, :], in_=w18.ap()[:, :])
            # w2 (pre-halved fp16): [128, (hc, O)]
            w2s = cpool.tile([128, NHC * O], F16)
            for hc in range(NHC):
                nc.sync.dma_start(
                    out=w2s[:, hc * O:(hc + 1) * O],
                    in_=w2h.ap()[hc * 128:(hc + 1) * 128, :],
                )
            b2s = cpool.tile([O, 1], F32)
            nc.sync.dma_start(out=b2s[:, :], in_=b2c.ap()[:, :])

            biasb = cpool.tile([128, 1], F32)
            nc.vector.memset(biasb[:, :], -EPS)

            v1 = cpool.tile([128, 256], F32)
            v2 = cpool.tile([O, BL], F32)

            s2f = cpool.tile([O, T * BL], F32)
            sg2f = cpool.tile([O, T * BL], F32)

            ps1_static = ps2_static = None
            if mode == "scan":
                ps1_static = ps1pool.tile([128, NHC * M], F32)
                nc.vector.memset(ps1_static[:, :], 0.125 * SC)
                ps2_static = ps2pool.tile([128, M], F32)
                nc.vector.memset(ps2_static[:, :], 0.125)

            rep_ctx = tc.For_i(0, reps, 1) if reps > 1 else contextlib.nullcontext()
            with rep_ctx:
                nc.vector.memset(v1[:, :], -2.0 * BETA)
                nc.vector.memset(v2[:, :], 1.0 - 2.0 * BETA)
                _body(nc, tc, locals(), mode)

            # ---- finalize: spk2 = 0.5*sigma2 + 0.5 ; mem2 = s2 + 1 ----
            spk8 = cpool.tile([O, T * BL], F8)
            mem16 = cpool.tile([O, T * BL], F16)
            nc.vector.tensor_scalar(
                out=spk8[:, :], in0=sg2f[:, :], scalar1=0.5, scalar2=0.5,
                op0=AL.mult, op1=AL.add,
            )
            nc.vector.tensor_scalar(
                out=mem16[:, :], in0=s2f[:, :], scalar1=1.0, scalar2=None,
                op0=AL.add,
            )
            nc.sync.dma_start(out=spk2o.ap()[:, :], in_=spk8[:, :])
            nc.sync.dma_start(out=mem2o.ap()[:, :], in_=mem16[:, :])

    nc.compile()
    return nc


def _body(nc, tc, env, mode="full"):
    (xhm, xhr, w1hs, w18s, w2s, b2s, biasb,
     v1, v2, s2f, sg2f,
     xhpool, x8pool, s1pool, sg1pool, ps1pool, ps2pool) = (
        env["xhm"], env["xhr"], env["w1hs"],
        env["w18s"], env["w2s"], env["b2s"],
        env["biasb"], env["v1"], env["v2"], env["s2f"],
        env["sg2f"], env["xhpool"], env["x8pool"], env["s1pool"],
        env["sg1pool"], env["ps1pool"], env["ps2pool"],
    )
    ps1_static, ps2_static = env["ps1_static"], env["ps2_static"]
    do_mm = mode in ("full", "mm", "mm1")
    do_mm2 = mode in ("full", "mm")
    do_scan = mode in ("full", "scan")

    def load(blk):
        """DMA x block + Act-cast xh->fp8 (descale 2^-6). Emitted TWO
        blocks ahead of the scan so the cast has a full block period of
        lead before mm1's first DR pass consumes it."""
        xbh = xhpool.tile([128, 7 * M], F16, tag="xbh")
        xb8 = x8pool.tile([128, 7 * M], F8, tag="xb8")
        nc.sync.dma_start(out=xbh[:, 0:NKC * M], in_=xhm.ap()[blk, :, :])
        nc.sync.dma_start(
            out=xbh[0:KREM, NKC * M:7 * M], in_=xhr.ap()[blk, :, :]
        )
        # fp8 image of xh for the w-residual term (2^-6 descale folded in)
        nc.scalar.activation(
            out=xb8[:, :], in_=xbh[:, :], func=AF.Copy, scale=1.0 / 64.0,
        )
        return xbh, xb8

    def mm1(bufs):
        """mm1 passes into a fresh PSUM tile, one block AHEAD of the
        scan (the in-order PE queue computes the next block's matmuls
        while the serial DVE scan runs)."""
        xbh, xb8 = bufs
        ps1 = ps1pool.tile([128, NHC * M], F32, tag="ps1")
        for hc in range(NHC):
            for kc in range(7):
                kp = 128 if kc < NKC else KREM
                nc.tensor.matmul(
                    ps1[:, hc * M:(hc + 1) * M],
                    lhsT=w1hs[0:kp, kc * H + hc * 128:kc * H + hc * 128 + 128],
                    rhs=xbh[0:kp, kc * M:(kc + 1) * M],
                    start=(kc == 0),
                    stop=False,
                )
            # xh8 @ wl8: adjacent k-chunk pairs (0,1)(2,3)(4,5) as fp8
            # DoubleRow, then the 17-row k-remainder as a plain fp8 pass.
            for j in range(3):
                lt = w18s[:, 2 * j * H:(2 * j + 2) * H].rearrange(
                    "p (i h) -> p i h", i=2
                )
                rt = xb8[:, 2 * j * M:(2 * j + 2) * M].rearrange(
                    "p (i m) -> p i m", i=2
                )
                nc.tensor.matmul(
                    ps1[:, hc * M:(hc + 1) * M],
                    lhsT=lt[:, :, hc * 128:hc * 128 + 128],
                    rhs=rt[:, :, :],
                    start=False,
                    stop=False,
                    perf_mode=mybir.MatmulPerfMode.DoubleRow,
                )
            nc.tensor.matmul(
                ps1[:, hc * M:(hc + 1) * M],
                lhsT=w18s[0:KREM, 6 * H + hc * 128:6 * H + hc * 128 + 128],
                rhs=xb8[0:KREM, 6 * M:7 * M],
                start=False,
                stop=True,
            )
        return ps1

    pend_l2 = [None]
    ps1_next = bufs_next = None
    if do_mm:
        bufs0 = load(0)
        bufs_next = load(1) if NB > 1 else None
        ps1_next = mm1(bufs0)
    for blk in range(NB):
        ps1 = ps1_next if do_mm else ps1_static
        if do_mm and blk + 2 < NB:
            bufs_nn = load(blk + 2)
        else:
            bufs_nn = None
        if do_mm and blk + 1 < NB:
            ps1_next = mm1(bufs_next)
        bufs_next = bufs_nn

        # ---- layer-1 scan over the TBLK timesteps (S = 4096*s) ----
        if do_scan:
            sg1 = sg1pool.tile([128, NHC * M], F16)
        else:
            sg1 = None
        ps1v = ps1.rearrange("p (c t b) -> p c t b", c=NHC, t=TBLK)
        sg1v = (sg1.rearrange("p (c t b) -> p c t b", c=NHC, t=TBLK)
                if sg1 is not None else None)
        v1v = v1.rearrange("p (c b) -> p c b", c=NHC)
        for t in range(TBLK) if do_scan else ():
            s1 = s1pool.tile([128, 256], F32)
            s1v = s1.rearrange("p (c b) -> p c b", c=NHC)
            nc.vector.scalar_tensor_tensor(
                out=s1v, in0=v1v, scalar=SC / 2.0, in1=ps1v[:, :, t, :],
                op0=AL.mult, op1=AL.add,
            )
            # sigma1 = (S > 0) * 2 in {0,2}; exact strict-> test, fp16 out
            nc.vector.tensor_scalar(
                out=sg1v[:, :, t, :], in0=s1v, scalar1=0.0, scalar2=2.0,
                op0=AL.is_gt, op1=AL.mult,
            )
            nc.vector.scalar_tensor_tensor(
                out=v1v, in0=s1v, scalar=2.0 * BETA / SC, in1=sg1v[:, :, t, :],
                op0=AL.mult, op1=AL.subtract,
            )

        # ---- mm2: cur2^T [O, M] in PSUM (single fp16 w2, no bias pass) ----
        if do_mm2:
            ps2 = ps2pool.tile([128, M], F32)
        elif mode == "mm1":
            ps2 = None
        else:
            ps2 = ps2_static
        for hc in range(NHC) if do_mm2 else ():
            nc.tensor.matmul(
                ps2[0:O, :],
                lhsT=w2s[:, hc * O:(hc + 1) * O],
                rhs=(sg1 if sg1 is not None else w1hs)[
                    :, hc * M:(hc + 1) * M],
                start=(hc == 0), stop=(hc == NHC - 1),
            )

        # ---- layer-2 scan, pipelined ONE BLOCK BEHIND so the DVE never
        #      waits for this block's mm2->Act staging round-trip ----
        if do_scan and ps2 is not None:
            if pend_l2[0] is not None:
                _scan_l2(nc, env, *pend_l2[0])
            c2s = s1pool.tile([O, M], F32, tag="c2s")
            # b2 + (beta - 1.5) rides the Act Identity's per-partition bias
            nc.scalar.activation(
                out=c2s[:, :], in_=ps2[0:O, :], func=AF.Identity,
                bias=b2s[0:O, 0:1], scale=1.0,
            )
            pend_l2[0] = (blk, c2s)
    if do_scan and pend_l2[0] is not None:
        _scan_l2(nc, env, *pend_l2[0])


def _scan_l2(nc, env, blk, c2s):
    v2, s2f, sg2f, biasb = env["v2"], env["s2f"], env["sg2f"], env["biasb"]
    for t in range(TBLK):
        g0 = (blk * TBLK + t) * BL
        s2sl = s2f[:, g0:g0 + BL]
        sg2sl = sg2f[:, g0:g0 + BL]
        nc.vector.scalar_tensor_tensor(
            out=s2sl, in0=v2[:, :], scalar=0.5,
            in1=c2s[:, t * BL:(t + 1) * BL],
            op0=AL.mult, op1=AL.add,
        )
        nc.scalar.activation(
            out=sg2sl, in_=s2sl, func=AF.Sign,
            bias=biasb[0:O, 0:1], scale=1.0,
        )
        nc.vector.scalar_tensor_tensor(
            out=v2[:, :], in0=s2sl, scalar=2.0 * BETA, in1=sg2sl,
            op0=AL.mult, op1=AL.subtract,
        )


def _f8(a):
    return a.astype(ml_dtypes.float8_e4m3)


def make_in_maps(x, w1, b1, w2, b2):
    """Host-side sharding + layout marshaling."""
    x = np.asarray(x, dtype=np.float32)
    w1 = np.asarray(w1, dtype=np.float32)
    b1 = np.asarray(b1, dtype=np.float32)
    w2 = np.asarray(w2, dtype=np.float32)
    b2 = np.asarray(b2, dtype=np.float32)

    w1T_aug = np.empty((KA, H), dtype=np.float32)
    w1T_aug[:I] = w1.T
    # x ships centered (u = x - 0.5): halves the fp16 ulp of the x-hi
    # stream; 0.5*colsum(w1) moves into the ones-row bias.
    w1T_aug[I] = b1 + (BETA - 1.0) + 0.5 * w1.T.sum(axis=0)
    wh16 = w1T_aug.astype(np.float16)                   # unscaled fp16 hi
    wl = w1T_aug - wh16.astype(np.float32)

    def pack_kchunks(rows, dtype):                      # [KA-ish, H] -> [128, 7H]
        out = np.zeros((128, 7, H), dtype=dtype)
        K_ = rows.shape[0]
        for c in range(7):
            n = min(128, K_ - c * 128)
            if n > 0:
                out[:n, c, :] = rows[c * 128:c * 128 + n]
        return out.reshape(128, 7 * H)

    w1h_in = pack_kchunks((wh16.astype(np.float32) * 64.0).astype(np.float16),
                          np.float16)
    w18_in = pack_kchunks(_f8(wl * SC), ml_dtypes.float8_e4m3)

    w2h_in = (np.ascontiguousarray(w2.T) * 0.5).astype(np.float16)  # [H, O]
    b2c_in = (b2 + (BETA - 1.5)).astype(np.float32).reshape(O, 1)

    in_maps = []
    for c in range(NCORES):
        xc = x[:, c * BL:(c + 1) * BL, :]               # [T, BL, I]
        xTc = np.empty((KA, T * BL), dtype=np.float32)
        xTc[:I] = xc.reshape(T * BL, I).T
        xTc[:I] -= 0.5                                  # centered (see above)
        xTc[I] = 1.0
        xh_sc = (xTc.astype(np.float16).astype(np.float32)
                 * 64.0).astype(np.float16)

        xh_b = xh_sc.reshape(KA, NB, M)
        xhm_in = np.ascontiguousarray(
            xh_b[:NKC * 128].reshape(NKC, 128, NB, M).transpose(2, 1, 0, 3)
        ).reshape(NB, 128, NKC * M)
        xhr_in = np.ascontiguousarray(
            xh_b[NKC * 128:].transpose(1, 0, 2)
        )                                               # [NB, 17, M]

        in_maps.append({
            "xhm": xhm_in, "xhr": xhr_in,
            "w1h": w1h_in, "w18": w18_in,
            "w2h": w2h_in, "b2c": b2c_in,
        })
    return in_maps


def gather_outputs(results):
    """results: list of per-core {'spk2': [O, T*BL] f8, 'mem2': [O, T*BL] f16}."""
    spk = np.empty((T, B, O), dtype=np.float32)
    mem = np.empty((T, B, O), dtype=np.float32)
    for c, r in enumerate(results):
        spk[:, c * BL:(c + 1) * BL, :] = (
            r["spk2"].astype(np.float32).reshape(O, T, BL).transpose(1, 2, 0)
        )
        mem[:, c * BL:(c + 1) * BL, :] = (
            r["mem2"].astype(np.float32).reshape(O, T, BL).transpose(1, 2, 0)
        )
    return spk, mem


_NC_CACHE = None


def kernel(x, w1, b1, w2, b2):
    global _NC_CACHE
    from concourse import bass_utils

    if _NC_CACHE is None:
        _NC_CACHE = build_nc()
    in_maps = make_in_maps(x, w1, b1, w2, b2)
    res = bass_utils.run_bass_kernel_spmd(
        _NC_CACHE, in_maps, core_ids=list(range(NCORES))
    )
    return gather_outputs(res.results)
